# revision 1
# baseline (speedup 1.0000x reference)
"""MoE gating (nn_MoEGate) Trainium2 Bass kernel.

kernel(x, W_g) -> (vals, idx), matching the jax reference:
    logits = x @ W_g.T ; scores = softmax(logits) ; vals, idx = top_k(scores, 8)

Strategy: data-parallel over the token dim — 8 NeuronCores each process a
4096-token shard with W_g replicated; no cross-core communication.

Per-core pipeline (T=4096 tokens, D=4096, E=64 experts, K=8):
  - x streamed in natural [token, d] layout (contiguous 2MB DMAs).
  - PE transposes 128x128 blocks of x (fp32, exact) into PSUM.
  - ACT rounds xT chunks PSUM->SBUF to float32r (x_r); DVE computes the
    f32r residual x_res = xT - x_r.
  - Gating streams x on the PE *moving* port (measured 4x cheaper on HW
    than x-stationary fp32 matmuls): two f32r matmuls per chunk against a
    single stationary [W_r || W_res] (W exactly split into two f32r
    planes), accumulating expert-major logitsT [2*64e, 512t] in PSUM.
    Error vs fp32: ~2e-7 (x = x_r + x_res up to 2^-22; W split exact).
  - Epilogue per group: ACT copies logitsT to SBUF; one fp32 PE matmul
    per 128-token tile against a stacked-identity fold matrix transposes
    AND folds the two halves: logits[t, e] = lgT[e, t] + lgT[64+e, t].
  - DVE iterative top-8 over token-major logits; masking by value-equality
    (exact fp32 ties are measure-zero; a tie would only drop a duplicate).
  - Softmax values: exp on ACT; sum + reciprocal + scale on DVE; index
    arithmetic on ACT.
  - Group schedule [4x7, 2, 1, 1] tiles shrinks the post-matmul topk tail.
"""
from contextlib import ExitStack

import numpy as np

import concourse.bacc as bacc
import concourse.mybir as mybir
import concourse.tile as tile
from concourse._compat import with_exitstack
from concourse.bass_utils import run_bass_kernel_spmd

F32 = mybir.dt.float32
F32R = mybir.dt.float32r
I32 = mybir.dt.int32
AX = mybir.AxisListType
ALU = mybir.AluOpType
EXP = mybir.ActivationFunctionType.Exp
COPY = mybir.ActivationFunctionType.Copy

N_CORES = 8
N_TOKENS = 32768
D = 4096
E = 64
K = 8
T = N_TOKENS // N_CORES  # 4096 tokens per core
NC = D // 128            # 32 d-chunks
IOBASE = 1087.0
SCHED_OVERRIDE = None    # debug hook: explicit [(n_tiles, flush), ...]
PMAJOR_OUT = True        # outputs are [128, T/128, K] partition-major
X_REUSE = False          # debug hook: skip x DMAs after group 0 (timing)


@with_exitstack
def _moe_gate_kernel(ctx: ExitStack, tc: tile.TileContext, outs, ins, n_tokens=T):
    nc = tc.nc
    x_d, w_d = ins
    vals_d, idx_d = outs

    consts = ctx.enter_context(tc.tile_pool(name="consts", bufs=1))
    psum = ctx.enter_context(tc.tile_pool(name="psum", bufs=1, space="PSUM"))

    ident128 = consts.tile([128, 128], F32, tag="ident128")
    nc.vector.memset(ident128[:], 1.0)
    nc.gpsimd.affine_select(
        ident128[:], ident128[:], pattern=[[-1, 128]], compare_op=ALU.is_equal,
        fill=0.0, base=0, channel_multiplier=1,
    )
    ident64 = consts.tile([64, 64], F32, tag="ident64")
    nc.vector.memset(ident64[:], 1.0)
    nc.gpsimd.affine_select(
        ident64[:], ident64[:], pattern=[[-1, 64]], compare_op=ALU.is_equal,
        fill=0.0, base=0, channel_multiplier=1,
    )
    # Fold matrix FM [128, 64]: FM[r, e] = (r == e) + (r == 64 + e); one
    # fp32 matmul lgT_block^T @ FM both transposes a 128-token block of
    # expert-major logitsT and folds the W_r/W_res halves.
    fm = consts.tile([128, E], F32, tag="fm")
    fm_b = consts.tile([128, E], F32, tag="fm_b")
    nc.vector.memset(fm[:], 1.0)
    nc.gpsimd.affine_select(
        fm[:], fm[:], pattern=[[-1, E]], compare_op=ALU.is_equal,
        fill=0.0, base=0, channel_multiplier=1,
    )
    nc.vector.memset(fm_b[:], 1.0)
    nc.gpsimd.affine_select(
        fm_b[:], fm_b[:], pattern=[[-1, E]], compare_op=ALU.is_equal,
        fill=0.0, base=-64, channel_multiplier=1,
    )
    nc.vector.tensor_tensor(fm[:], fm[:], fm_b[:], op=ALU.add)

    iorev = consts.tile([128, E], F32, tag="iorev")
    nc.gpsimd.iota(
        iorev[:], pattern=[[-1, E]], base=int(IOBASE), channel_multiplier=0,
        allow_small_or_imprecise_dtypes=True,
    )
    negbig = consts.tile([128, 1], F32, tag="negbig")
    nc.vector.memset(negbig[:], -30000.0)

    # W split: wcat[128 d, c, 0:64] = f32r(W^T chunk), [.., 64:128] = f32r
    # residual — W_r + W_res == W up to ~2^-24.
    wcat = consts.tile([128, NC, 2 * E], F32R, tag="wcat")
    with tc.tile_pool(name="wsetup", bufs=1) as wsp:
        w_sb = wsp.tile([64, D], F32, tag="w_sb")
        for q in range(4):
            nc.scalar.dma_start(
                w_sb[:, q * (D // 4):(q + 1) * (D // 4)],
                w_d[:, q * (D // 4):(q + 1) * (D // 4)],
            )
        wt_sb = wsp.tile([128, NC, E], F32, tag="wt_sb")
        for cq in range(NC // 8):
            wt_ps = psum.tile([128, 512], F32, tag="xt", bufs=4, name="wt_ps")
            for c8 in range(8):
                c = cq * 8 + c8
                nc.tensor.transpose(
                    wt_ps[:, c8 * 64:(c8 + 1) * 64],
                    w_sb[:, c * 128:(c + 1) * 128],
                    ident64[:],
                )
            nc.vector.tensor_copy(wt_sb[:, cq * 8:(cq + 1) * 8, :], wt_ps[:])
        wt_r = wsp.tile([128, NC, E], F32R, tag="wt_r")
        nc.vector.tensor_copy(wt_r[:], wt_sb[:])
        wres = wsp.tile([128, NC, E], F32R, tag="wres")
        nc.vector.tensor_tensor(wres[:], wt_sb[:], wt_r[:].bitcast(F32),
                                op=ALU.subtract)
        nc.vector.tensor_copy(wcat[:, :, 0:E], wt_r[:])
        nc.vector.tensor_copy(wcat[:, :, E:2 * E], wres[:])

    sb = ctx.enter_context(tc.tile_pool(name="main", bufs=1))

    def topk_batch_gen(lp, tile0, G):
        # lp: [128, G, E] logits for token-tiles [tile0, tile0+G).
        # Generator: emission is woven into the next group's chunk loop so
        # the multi-us DVE blob doesn't sit ahead of that group's x_res
        # subtracts in the in-order DVE queue.
        lp_flat = lp[:].rearrange("p g e -> p (g e)")
        exp_t = sb.tile([128, G * E], F32, tag="exp" + str(G), bufs=2, name="exp_t")
        nc.scalar.activation(exp_t[:], lp_flat, EXP)
        den = sb.tile([128, G], F32, tag="den" + str(G), bufs=2, name="den")
        nc.vector.reduce_sum(
            den[:], exp_t[:].rearrange("p (g e) -> p g e", g=G), axis=AX.X
        )
        rden = sb.tile([128, G], F32, tag="rden" + str(G), bufs=2, name="rden")
        nc.vector.reciprocal(rden[:], den[:])
        yield

        ws = lp[:, :, :]
        mxs = sb.tile([128, G, K], F32, tag="mxs" + str(G), bufs=2, name="mxs")
        rixs = sb.tile([128, G, K], F32, tag="rixs" + str(G), bufs=2, name="rixs")
        eq = sb.tile([128, G, E], mybir.dt.uint8, tag="eq" + str(G), bufs=2, name="eq")
        im = sb.tile([128, G, E], F32, tag="im" + str(G), bufs=2, name="im")
        io_b = iorev[:].unsqueeze(1).broadcast_to([128, G, E])
        nb_flat = negbig[:].broadcast_to([128, G * E])
        ws_flat = lp[:].rearrange("p g e -> p (g e)")
        for k in range(K):
            nc.vector.reduce_max(mxs[:, :, k], ws, axis=AX.X)
            mx_b = mxs[:, :, k].unsqueeze(2).broadcast_to([128, G, E])
            nc.vector.tensor_tensor(eq[:], ws, mx_b, op=ALU.is_ge)
            nc.vector.tensor_tensor(im[:], eq[:], io_b, op=ALU.mult)
            nc.vector.tensor_reduce(rixs[:, :, k], im[:], axis=AX.X, op=ALU.max)
            if k < K - 1:
                # mask by value-equality (eq): exact fp32 ties are
                # measure-zero; a tie would only drop a duplicate value.
                nc.vector.copy_predicated(
                    ws_flat, eq[:].rearrange("p g e -> p (g e)"), nb_flat
                )
            yield

        exk = sb.tile([128, G, K], F32, tag="exk" + str(G), bufs=2, name="exk")
        nc.scalar.activation(exk[:], mxs[:], EXP)
        vals = sb.tile([128, G, K], F32, tag="vals" + str(G), bufs=2, name="vals")
        rden_b = rden[:].unsqueeze(2).broadcast_to([128, G, K])
        nc.vector.tensor_tensor(vals[:], exk[:], rden_b, op=ALU.mult)
        idxf = sb.tile([128, G, K], F32, tag="idxf" + str(G), bufs=2, name="idxf")
        nc.scalar.activation(idxf[:], rixs[:], COPY, bias=IOBASE, scale=-1.0)
        idxi = sb.tile([128, G, K], I32, tag="idxi" + str(G), bufs=2, name="idxi")
        nc.scalar.copy(idxi[:], idxf[:])

        # outputs are partition-major [128, T/128, K] in DRAM (contiguous
        # 32B*G runs per partition, descriptor-efficient); host un-permutes.
        nc.scalar.dma_start(vals_d[:, tile0:tile0 + G, :], vals[:])
        nc.scalar.dma_start(idx_d[:, tile0:tile0 + G, :], idxi[:])
        yield

    # Group schedule: (n_tiles, flush). First six groups pair into 8-tile
    # topk batches (efficient shape); the tail shrinks to 2/1/1-tile groups
    # so the final topk batch after the last matmul is as small as possible.
    if SCHED_OVERRIDE is not None:
        SCHED = SCHED_OVERRIDE
    elif n_tokens == 4096:
        SCHED = [(4, False), (4, True), (4, False), (4, True), (4, False),
                 (4, True), (4, True), (2, True), (1, True), (1, True)]
    else:
        SCHED = []
        rem = n_tokens // 128
        while rem:
            gsz = min(4, rem)
            SCHED.append((gsz, True))
            rem -= gsz
    assert sum(s for s, _ in SCHED) * 128 == n_tokens

    lp = None
    pending = None   # in-flight topk emission generator (woven)
    _DONE = object()
    lp_pos = 0       # tiles already copied into the current lp batch
    batch_tile0 = 0  # first global tile index of the current batch
    tile0 = 0        # global tile index of the current group's first tile
    for g, (NT, flush) in enumerate(SCHED):
        W_ = NT * 128  # active token columns this group
        x_tiles = []
        if X_REUSE and g > 0:
            x_tiles = _x_keep[:NT]
        else:
            # Split loads along d so transposes wait on sub-MB DMA quanta
            # instead of whole 2MB tiles (DMA/PE overlap on HW).
            n_splits = 8 if g == 0 else (4 if g == 1 else 2)
            for j in range(NT):
                xt = sb.tile([128, D], F32, tag="x", bufs=8, name=f"x_{g}_{j}")
                x_tiles.append(xt)
            dq = D // n_splits
            for q in range(n_splits):
                for j in range(NT):
                    r0 = (tile0 + j) * 128
                    nc.sync.dma_start(
                        x_tiles[j][:, q * dq:(q + 1) * dq],
                        x_d[r0:r0 + 128, q * dq:(q + 1) * dq],
                    )
            if g == 0:
                _x_keep = list(x_tiles)

        # expert-major logitsT accumulator: rows 0:64 = W_r part, 64:128 =
        # W_res part. One bank; bufs=2 decouples consecutive groups.
        lgT_ps = psum.tile([128, 512], F32, tag="lgT", bufs=2, name="lgT_ps")

        def gating(c, bufs_pair):
            xt_r_c, xres_c = bufs_pair
            nc.tensor.matmul(
                lgT_ps[:, 0:W_], wcat[:, c, :], xt_r_c[:, 0:W_],
                start=(c == 0), stop=False, skip_group_check=True,
            )
            nc.tensor.matmul(
                lgT_ps[:, 0:W_], wcat[:, c, :], xres_c[:, 0:W_],
                start=False, stop=(c == NC - 1), skip_group_check=True,
            )

        # software pipeline: gating for chunk c-SKEW is emitted between the
        # transposes of chunk c so the in-order PE stream never waits on the
        # ACT/DVE production of its own chunk's x_r/x_res.
        SKEW = 3
        xt_bufs = {}
        for c in range(NC):
            xt_ps = psum.tile([128, 512], F32, tag="xt", bufs=4, name="xt_ps")
            for j in range(NT):
                nc.tensor.transpose(
                    xt_ps[:, j * 128:(j + 1) * 128],
                    x_tiles[j][:, c * 128:(c + 1) * 128],
                    ident128[:],
                )
            xt_r = sb.tile([128, 512], F32R, tag="xtr", bufs=5, name="xt_r")
            nc.scalar.copy(xt_r[:, 0:W_], xt_ps[:, 0:W_])
            xres = sb.tile([128, 512], F32R, tag="xrs", bufs=5, name="xres")
            nc.vector.tensor_tensor(
                xres[:, 0:W_], xt_ps[:, 0:W_], xt_r[:, 0:W_].bitcast(F32),
                op=ALU.subtract,
            )
            xt_bufs[c] = (xt_r, xres)
            if c >= SKEW:
                gating(c - SKEW, xt_bufs.pop(c - SKEW))
            if pending is not None and c % 3 == 2:
                if next(pending, _DONE) is _DONE:
                    pending = None
        for c in range(NC - SKEW, NC):
            gating(c, xt_bufs.pop(c))
        while pending is not None:
            if next(pending, _DONE) is _DONE:
                pending = None

        # Epilogue: logitsT -> SBUF (ACT), then per-tile fold+transpose
        # matmuls into rotating single-group PSUM banks.
        lgT_sb = sb.tile([128, 512], F32, tag="lgTsb", bufs=2, name="lgT_sb")
        nc.scalar.copy(lgT_sb[:, 0:W_], lgT_ps[:, 0:W_])

        if lp is None:
            bsz = NT if flush else NT + 4  # batch size: this group (+pair)
            lp = sb.tile([128, bsz, E], F32, tag=f"lp{bsz}", bufs=2,
                         name=f"lp{bsz}")
            lp_pos = 0
            batch_tile0 = tile0
        for j in range(NT):
            fold_ps = psum.tile([128, E], F32, tag="fold", bufs=2,
                                name="fold_ps")
            nc.tensor.matmul(
                fold_ps[:, :], lgT_sb[:, j * 128:(j + 1) * 128], fm[:],
                start=True, stop=True,
            )
            nc.scalar.copy(lp[:, lp_pos + j, :], fold_ps[:, :])
        lp_pos += NT
        if flush:
            pending = topk_batch_gen(lp, batch_tile0, lp_pos)
            next(pending)  # emit exp/den/recip immediately
            lp = None
        tile0 += NT
    while pending is not None:
        if next(pending, _DONE) is _DONE:
            pending = None


_MODEL_CACHE = {}


def build_model(n_tokens=T):
    if n_tokens in _MODEL_CACHE:
        return _MODEL_CACHE[n_tokens]
    nc = bacc.Bacc(
        "TRN2",
        target_bir_lowering=False,
        debug=False,
        enable_asserts=False,
        num_devices=N_CORES,
    )
    x_d = nc.dram_tensor("x", [n_tokens, D], F32, kind="ExternalInput").ap()
    w_d = nc.dram_tensor("w", [E, D], F32, kind="ExternalInput").ap()
    vals_d = nc.dram_tensor(
        "vals", [128, n_tokens // 128, K], F32, kind="ExternalOutput").ap()
    idx_d = nc.dram_tensor(
        "idx", [128, n_tokens // 128, K], I32, kind="ExternalOutput").ap()
    with tile.TileContext(nc) as tc:
        _moe_gate_kernel(tc, [vals_d, idx_d], [x_d, w_d], n_tokens=n_tokens)
    nc.compile()
    _MODEL_CACHE[n_tokens] = nc
    return nc


def run_on_cores(x, W_g, trace=False, trace_kwargs=None):
    """x [32768, 4096] f32, W_g [64, 4096] f32 -> (vals, idx), plus results obj."""
    nc = build_model()
    x = np.ascontiguousarray(np.asarray(x, dtype=np.float32))
    W_g = np.ascontiguousarray(np.asarray(W_g, dtype=np.float32))
    shards = np.split(x, N_CORES, axis=0)
    in_maps = [{"x": shards[i], "w": W_g} for i in range(N_CORES)]
    res = run_bass_kernel_spmd(
        nc, in_maps, core_ids=list(range(N_CORES)), trace=trace,
        **(trace_kwargs or {}),
    )
    # device layout is [128, T/128, K] partition-major; token = tile*128 + p
    vals = np.concatenate(
        [r["vals"].transpose(1, 0, 2).reshape(T, K) for r in res.results], axis=0)
    idx = np.concatenate(
        [r["idx"].transpose(1, 0, 2).reshape(T, K) for r in res.results], axis=0)
    return (vals, idx), res


def kernel(x, W_g):
    (vals, idx), _ = run_on_cores(x, W_g)
    return vals, idx



# revision 2
# speedup vs baseline: 1.2261x; 1.2261x over previous
"""MoE gating (nn_MoEGate) Trainium2 Bass kernel.

kernel(x, W_g) -> (vals, idx), matching the jax reference:
    logits = x @ W_g.T ; scores = softmax(logits) ; vals, idx = top_k(scores, 8)

Strategy: data-parallel over the token dim — 8 NeuronCores each process a
4096-token shard with W_g replicated; no cross-core communication.

Per-core pipeline (T=4096 tokens, D=4096, E=64 experts, K=8):
  - x streamed in natural [token, d] layout (contiguous 2MB DMAs).
  - PE transposes 128x128 blocks of x (fp32, exact) into PSUM.
  - ACT rounds xT chunks PSUM->SBUF to float32r (x_r); DVE computes the
    f32r residual x_res = xT - x_r.
  - Gating streams x on the PE *moving* port: two f32r matmuls per chunk
    against a single stationary [W_r || W_res] (W exactly split into two
    f32r planes), accumulating expert-major logitsT [2*64e, 512t] in PSUM.
    Error vs fp32: ~2e-7 (x = x_r + x_res up to 2^-22; W split exact).
  - Epilogue per group: ACT copies logitsT to SBUF; one fp32 PE matmul
    per 128-token tile against a stacked-identity fold matrix transposes
    AND folds the two halves: logits[t, e] = lgT[e, t] + lgT[64+e, t].
  - Top-8 via the DVE Max8 datapath: one `max` (top-8 values, sorted
    descending) + one `max_index` (their indices) per 128-token tile —
    replaces an iterative mask loop; ties resolve to ascending indices,
    matching lax.top_k's stable order.
  - Softmax: ACT exp with fused free-dim accumulation (accum_out) gives
    the denominator in the same instruction; DVE reciprocal + scale.
  - Group schedule [4x7, 2, 1, 1] tiles shrinks the post-matmul tail.
"""
from contextlib import ExitStack

import numpy as np

import concourse.bacc as bacc
import concourse.mybir as mybir
import concourse.tile as tile
from concourse._compat import with_exitstack
from concourse.bass_utils import run_bass_kernel_spmd

F32 = mybir.dt.float32
F32R = mybir.dt.float32r
I32 = mybir.dt.int32
U32 = mybir.dt.uint32
AX = mybir.AxisListType
ALU = mybir.AluOpType
EXP = mybir.ActivationFunctionType.Exp
COPY = mybir.ActivationFunctionType.Copy

N_CORES = 8
N_TOKENS = 32768
D = 4096
E = 64
K = 8
T = N_TOKENS // N_CORES  # 4096 tokens per core
NC = D // 128            # 32 d-chunks
SCHED_OVERRIDE = None    # debug hook: explicit [(n_tiles, flush), ...]
X_REUSE = False          # debug hook: skip x DMAs after group 0 (timing)


@with_exitstack
def _moe_gate_kernel(ctx: ExitStack, tc: tile.TileContext, outs, ins, n_tokens=T):
    nc = tc.nc
    x_d, w_d = ins
    vals_d, idx_d = outs

    consts = ctx.enter_context(tc.tile_pool(name="consts", bufs=1))
    psum = ctx.enter_context(tc.tile_pool(name="psum", bufs=1, space="PSUM"))

    ident128 = consts.tile([128, 128], F32, tag="ident128")
    nc.vector.memset(ident128[:], 1.0)
    nc.gpsimd.affine_select(
        ident128[:], ident128[:], pattern=[[-1, 128]], compare_op=ALU.is_equal,
        fill=0.0, base=0, channel_multiplier=1,
    )
    ident64 = consts.tile([64, 64], F32, tag="ident64")
    nc.vector.memset(ident64[:], 1.0)
    nc.gpsimd.affine_select(
        ident64[:], ident64[:], pattern=[[-1, 64]], compare_op=ALU.is_equal,
        fill=0.0, base=0, channel_multiplier=1,
    )
    # Fold matrix FM [128, 64]: FM[r, e] = (r == e) + (r == 64 + e); one
    # fp32 matmul lgT_block^T @ FM both transposes a 128-token block of
    # expert-major logitsT and folds the W_r/W_res halves.
    fm = consts.tile([128, E], F32, tag="fm")
    fm_b = consts.tile([128, E], F32, tag="fm_b")
    nc.vector.memset(fm[:], 1.0)
    nc.gpsimd.affine_select(
        fm[:], fm[:], pattern=[[-1, E]], compare_op=ALU.is_equal,
        fill=0.0, base=0, channel_multiplier=1,
    )
    nc.vector.memset(fm_b[:], 1.0)
    nc.gpsimd.affine_select(
        fm_b[:], fm_b[:], pattern=[[-1, E]], compare_op=ALU.is_equal,
        fill=0.0, base=-64, channel_multiplier=1,
    )
    nc.vector.tensor_tensor(fm[:], fm[:], fm_b[:], op=ALU.add)

    # W split: wcat[128 d, c, 0:64] = f32r(W^T chunk), [.., 64:128] = f32r
    # residual — W_r + W_res == W up to ~2^-24.
    wcat = consts.tile([128, NC, 2 * E], F32R, tag="wcat")
    with tc.tile_pool(name="wsetup", bufs=1) as wsp:
        w_sb = wsp.tile([64, D], F32, tag="w_sb")
        for q in range(4):
            nc.scalar.dma_start(
                w_sb[:, q * (D // 4):(q + 1) * (D // 4)],
                w_d[:, q * (D // 4):(q + 1) * (D // 4)],
            )
        wt_sb = wsp.tile([128, NC, E], F32, tag="wt_sb")
        for cq in range(NC // 8):
            wt_ps = psum.tile([128, 512], F32, tag="xt", bufs=4, name="wt_ps")
            for c8 in range(8):
                c = cq * 8 + c8
                nc.tensor.transpose(
                    wt_ps[:, c8 * 64:(c8 + 1) * 64],
                    w_sb[:, c * 128:(c + 1) * 128],
                    ident64[:],
                )
            nc.vector.tensor_copy(wt_sb[:, cq * 8:(cq + 1) * 8, :], wt_ps[:])
        wt_r = wsp.tile([128, NC, E], F32R, tag="wt_r")
        nc.vector.tensor_copy(wt_r[:], wt_sb[:])
        wres = wsp.tile([128, NC, E], F32R, tag="wres")
        nc.vector.tensor_tensor(wres[:], wt_sb[:], wt_r[:].bitcast(F32),
                                op=ALU.subtract)
        nc.vector.tensor_copy(wcat[:, :, 0:E], wt_r[:])
        nc.vector.tensor_copy(wcat[:, :, E:2 * E], wres[:])

    sb = ctx.enter_context(tc.tile_pool(name="main", bufs=1))

    def topk_batch_gen(lp, tile0, G):
        # lp: [128, G, E] logits for token-tiles [tile0, tile0+G).
        # Generator: emission is woven into the next group's chunk loop so
        # the DVE/ACT epilogue ops don't sit ahead of that group's x_res
        # subtracts in the in-order engine queues.
        exp_t = sb.tile([128, G, E], F32, tag="exp" + str(G), bufs=2, name="exp_t")
        den = sb.tile([128, G], F32, tag="den" + str(G), bufs=2, name="den")
        for j in range(G):
            # exp of all 64 logits with fused free-dim sum -> denominator
            nc.scalar.activation(exp_t[:, j, :], lp[:, j, :], EXP,
                                 accum_out=den[:, j:j + 1])
        yield

        # Hardware top-8: Max8 datapath returns the 8 largest values per
        # partition in descending order; max_index recovers their indices
        # (duplicate values get distinct, ascending indices).
        mxs = sb.tile([128, G, K], F32, tag="mxs" + str(G), bufs=2, name="mxs")
        idxi = sb.tile([128, G, K], U32, tag="idxi" + str(G), bufs=2, name="idxi")
        for j in range(G):
            nc.vector.max(mxs[:, j, :], lp[:, j, :])
            nc.vector.max_index(idxi[:, j, :], mxs[:, j, :], lp[:, j, :])
            if j % 3 == 2:
                yield

        rden = sb.tile([128, G], F32, tag="rden" + str(G), bufs=2, name="rden")
        nc.vector.reciprocal(rden[:], den[:])
        exk = sb.tile([128, G, K], F32, tag="exk" + str(G), bufs=2, name="exk")
        nc.scalar.activation(exk[:], mxs[:], EXP)
        vals = sb.tile([128, G, K], F32, tag="vals" + str(G), bufs=2, name="vals")
        rden_b = rden[:].unsqueeze(2).broadcast_to([128, G, K])
        nc.vector.tensor_tensor(vals[:], exk[:], rden_b, op=ALU.mult)
        yield

        # outputs are partition-major [128, T/128, K] in DRAM (contiguous
        # 32B*G runs per partition, descriptor-efficient); host un-permutes.
        nc.scalar.dma_start(vals_d[:, tile0:tile0 + G, :], vals[:])
        nc.scalar.dma_start(idx_d[:, tile0:tile0 + G, :], idxi[:].bitcast(I32))
        yield

    # Group schedule: (n_tiles, flush). First six groups pair into 8-tile
    # topk batches (efficient shape); the tail shrinks to 2/1/1-tile groups
    # so the final topk batch after the last matmul is as small as possible.
    if SCHED_OVERRIDE is not None:
        SCHED = SCHED_OVERRIDE
    elif n_tokens == 4096:
        SCHED = [(4, False), (4, True), (4, False), (4, True), (4, False),
                 (4, True), (4, True), (2, True), (1, True), (1, True)]
    else:
        SCHED = []
        rem = n_tokens // 128
        while rem:
            gsz = min(4, rem)
            SCHED.append((gsz, True))
            rem -= gsz
    assert sum(s for s, _ in SCHED) * 128 == n_tokens

    lp = None
    pending = None   # in-flight topk emission generator (woven)
    _DONE = object()
    lp_pos = 0       # tiles already copied into the current lp batch
    batch_tile0 = 0  # first global tile index of the current batch
    tile0 = 0        # global tile index of the current group's first tile
    for g, (NT, flush) in enumerate(SCHED):
        W_ = NT * 128  # active token columns this group
        x_tiles = []
        if X_REUSE and g > 0:
            x_tiles = _x_keep[:NT]
        else:
            # Split loads along d so transposes wait on sub-MB DMA quanta
            # instead of whole 2MB tiles (DMA/PE overlap on HW).
            n_splits = 8 if g == 0 else (4 if g == 1 else 2)
            for j in range(NT):
                xt = sb.tile([128, D], F32, tag="x", bufs=8, name=f"x_{g}_{j}")
                x_tiles.append(xt)
            dq = D // n_splits
            for q in range(n_splits):
                for j in range(NT):
                    r0 = (tile0 + j) * 128
                    nc.sync.dma_start(
                        x_tiles[j][:, q * dq:(q + 1) * dq],
                        x_d[r0:r0 + 128, q * dq:(q + 1) * dq],
                    )
            if g == 0:
                _x_keep = list(x_tiles)

        # expert-major logitsT accumulator: rows 0:64 = W_r part, 64:128 =
        # W_res part. One bank; bufs=2 decouples consecutive groups.
        lgT_ps = psum.tile([128, 512], F32, tag="lgT", bufs=2, name="lgT_ps")

        def gating(c, bufs_pair):
            xt_r_c, xres_c = bufs_pair
            nc.tensor.matmul(
                lgT_ps[:, 0:W_], wcat[:, c, :], xt_r_c[:, 0:W_],
                start=(c == 0), stop=False, skip_group_check=True,
            )
            nc.tensor.matmul(
                lgT_ps[:, 0:W_], wcat[:, c, :], xres_c[:, 0:W_],
                start=False, stop=(c == NC - 1), skip_group_check=True,
            )

        # software pipeline: gating for chunk c-SKEW is emitted between the
        # transposes of chunk c so the in-order PE stream never waits on the
        # ACT/DVE production of its own chunk's x_r/x_res.
        SKEW = 3
        xt_bufs = {}
        for c in range(NC):
            xt_ps = psum.tile([128, 512], F32, tag="xt", bufs=4, name="xt_ps")
            for j in range(NT):
                nc.tensor.transpose(
                    xt_ps[:, j * 128:(j + 1) * 128],
                    x_tiles[j][:, c * 128:(c + 1) * 128],
                    ident128[:],
                )
            xt_r = sb.tile([128, 512], F32R, tag="xtr", bufs=5, name="xt_r")
            nc.scalar.copy(xt_r[:, 0:W_], xt_ps[:, 0:W_])
            xres = sb.tile([128, 512], F32R, tag="xrs", bufs=5, name="xres")
            nc.vector.tensor_tensor(
                xres[:, 0:W_], xt_ps[:, 0:W_], xt_r[:, 0:W_].bitcast(F32),
                op=ALU.subtract,
            )
            xt_bufs[c] = (xt_r, xres)
            if c >= SKEW:
                gating(c - SKEW, xt_bufs.pop(c - SKEW))
            if pending is not None and c % 3 == 2:
                if next(pending, _DONE) is _DONE:
                    pending = None
        for c in range(NC - SKEW, NC):
            gating(c, xt_bufs.pop(c))
        while pending is not None:
            if next(pending, _DONE) is _DONE:
                pending = None

        # Epilogue: logitsT -> SBUF (ACT), then per-tile fold+transpose
        # matmuls into rotating single-group PSUM banks.
        lgT_sb = sb.tile([128, 512], F32, tag="lgTsb", bufs=2, name="lgT_sb")
        nc.scalar.copy(lgT_sb[:, 0:W_], lgT_ps[:, 0:W_])

        if lp is None:
            bsz = NT if flush else NT + 4  # batch size: this group (+pair)
            lp = sb.tile([128, bsz, E], F32, tag=f"lp{bsz}", bufs=2,
                         name=f"lp{bsz}")
            lp_pos = 0
            batch_tile0 = tile0
        for j in range(NT):
            fold_ps = psum.tile([128, E], F32, tag="fold", bufs=2,
                                name="fold_ps")
            nc.tensor.matmul(
                fold_ps[:, :], lgT_sb[:, j * 128:(j + 1) * 128], fm[:],
                start=True, stop=True,
            )
            nc.scalar.copy(lp[:, lp_pos + j, :], fold_ps[:, :])
        lp_pos += NT
        if flush:
            pending = topk_batch_gen(lp, batch_tile0, lp_pos)
            next(pending)  # emit exp/den immediately
            lp = None
        tile0 += NT
    while pending is not None:
        if next(pending, _DONE) is _DONE:
            pending = None


_MODEL_CACHE = {}


def build_model(n_tokens=T):
    if n_tokens in _MODEL_CACHE:
        return _MODEL_CACHE[n_tokens]
    nc = bacc.Bacc(
        "TRN2",
        target_bir_lowering=False,
        debug=False,
        enable_asserts=False,
        num_devices=N_CORES,
    )
    x_d = nc.dram_tensor("x", [n_tokens, D], F32, kind="ExternalInput").ap()
    w_d = nc.dram_tensor("w", [E, D], F32, kind="ExternalInput").ap()
    vals_d = nc.dram_tensor(
        "vals", [128, n_tokens // 128, K], F32, kind="ExternalOutput").ap()
    idx_d = nc.dram_tensor(
        "idx", [128, n_tokens // 128, K], I32, kind="ExternalOutput").ap()
    with tile.TileContext(nc) as tc:
        _moe_gate_kernel(tc, [vals_d, idx_d], [x_d, w_d], n_tokens=n_tokens)
    nc.compile()
    _MODEL_CACHE[n_tokens] = nc
    return nc


def run_on_cores(x, W_g, trace=False, trace_kwargs=None):
    """x [32768, 4096] f32, W_g [64, 4096] f32 -> (vals, idx), plus results obj."""
    nc = build_model()
    x = np.ascontiguousarray(np.asarray(x, dtype=np.float32))
    W_g = np.ascontiguousarray(np.asarray(W_g, dtype=np.float32))
    shards = np.split(x, N_CORES, axis=0)
    in_maps = [{"x": shards[i], "w": W_g} for i in range(N_CORES)]
    res = run_bass_kernel_spmd(
        nc, in_maps, core_ids=list(range(N_CORES)), trace=trace,
        **(trace_kwargs or {}),
    )
    # device layout is [128, T/128, K] partition-major; token = tile*128 + p
    vals = np.concatenate(
        [r["vals"].transpose(1, 0, 2).reshape(T, K) for r in res.results], axis=0)
    idx = np.concatenate(
        [r["idx"].transpose(1, 0, 2).reshape(T, K) for r in res.results], axis=0)
    return (vals, idx), res


def kernel(x, W_g):
    (vals, idx), _ = run_on_cores(x, W_g)
    return vals, idx


# revision 7
# speedup vs baseline: 1.3282x; 1.0832x over previous
"""MoE gating (nn_MoEGate) Trainium2 Bass kernel.

kernel(x, W_g) -> (vals, idx), matching the jax reference:
    logits = x @ W_g.T ; scores = softmax(logits) ; vals, idx = top_k(scores, 8)

Strategy: data-parallel over the token dim — 8 NeuronCores each process a
4096-token shard with W_g replicated; no cross-core communication.

Per-core pipeline (T=4096 tokens, D=4096, E=64 experts, K=8):
  - x streamed in natural [token, d] layout (contiguous 2MB DMAs).
  - PE transposes 128x128 blocks of x (fp32, exact) into PSUM.
  - ACT rounds xT chunks PSUM->SBUF to float32r (x_r); DVE computes the
    f32r residual x_res = xT - x_r.
  - Gating streams x on the PE *moving* port: two f32r matmuls per chunk
    against a single stationary [W_r || W_res] (W exactly split into two
    f32r planes), accumulating expert-major logitsT [2*64e, 512t] in PSUM.
    Error vs fp32: ~2e-7 (x = x_r + x_res up to 2^-22; W split exact).
  - Epilogue per group: ACT copies logitsT to SBUF; one fp32 PE matmul
    per 128-token tile against a stacked-identity fold matrix transposes
    AND folds the two halves: logits[t, e] = lgT[e, t] + lgT[64+e, t].
  - Top-8 via the DVE Max8 datapath: one `max` (top-8 values, sorted
    descending) + one `max_index` (their indices) per 128-token tile —
    replaces an iterative mask loop; ties resolve to ascending indices,
    matching lax.top_k's stable order.
  - Softmax: ACT exp with fused free-dim accumulation (accum_out) gives
    the denominator in the same instruction; DVE reciprocal + scale.
  - Group schedule [4x7, 2, 1, 1] tiles shrinks the post-matmul tail.
"""
from contextlib import ExitStack

import numpy as np

import concourse.bacc as bacc
import concourse.mybir as mybir
import concourse.tile as tile
from concourse._compat import with_exitstack
from concourse.bass_utils import run_bass_kernel_spmd

F32 = mybir.dt.float32
F32R = mybir.dt.float32r
I32 = mybir.dt.int32
U32 = mybir.dt.uint32
AX = mybir.AxisListType
ALU = mybir.AluOpType
EXP = mybir.ActivationFunctionType.Exp
COPY = mybir.ActivationFunctionType.Copy

N_CORES = 8
N_TOKENS = 32768
D = 4096
E = 64
K = 8
T = N_TOKENS // N_CORES  # 4096 tokens per core
NC = D // 128            # 32 d-chunks
SCHED_OVERRIDE = None    # debug hook: explicit [(n_tiles, flush), ...]
X_REUSE = False          # debug hook: skip x DMAs after group 0 (timing)


@with_exitstack
def _moe_gate_kernel(ctx: ExitStack, tc: tile.TileContext, outs, ins, n_tokens=T):
    nc = tc.nc
    x_d, w_d = ins
    vals_d, idx_d = outs

    consts = ctx.enter_context(tc.tile_pool(name="consts", bufs=1))
    psum = ctx.enter_context(tc.tile_pool(name="psum", bufs=1, space="PSUM"))

    ident128 = consts.tile([128, 128], F32, tag="ident128")
    nc.vector.memset(ident128[:], 1.0)
    nc.gpsimd.affine_select(
        ident128[:], ident128[:], pattern=[[-1, 128]], compare_op=ALU.is_equal,
        fill=0.0, base=0, channel_multiplier=1,
    )
    ident64 = consts.tile([64, 64], F32, tag="ident64")
    nc.vector.memset(ident64[:], 1.0)
    nc.gpsimd.affine_select(
        ident64[:], ident64[:], pattern=[[-1, 64]], compare_op=ALU.is_equal,
        fill=0.0, base=0, channel_multiplier=1,
    )
    # Fold matrix FM [128, 64]: FM[r, e] = (r == e) + (r == 64 + e); one
    # fp32 matmul lgT_block^T @ FM both transposes a 128-token block of
    # expert-major logitsT and folds the W_r/W_res halves.
    fm = consts.tile([128, E], F32, tag="fm")
    fm_b = consts.tile([128, E], F32, tag="fm_b")
    nc.vector.memset(fm[:], 1.0)
    nc.gpsimd.affine_select(
        fm[:], fm[:], pattern=[[-1, E]], compare_op=ALU.is_equal,
        fill=0.0, base=0, channel_multiplier=1,
    )
    nc.vector.memset(fm_b[:], 1.0)
    nc.gpsimd.affine_select(
        fm_b[:], fm_b[:], pattern=[[-1, E]], compare_op=ALU.is_equal,
        fill=0.0, base=-64, channel_multiplier=1,
    )
    nc.vector.tensor_tensor(fm[:], fm[:], fm_b[:], op=ALU.add)

    sb = ctx.enter_context(tc.tile_pool(name="main", bufs=1))

    # W split: wcat[128 d, c, 0:64] = f32r(W^T chunk), [.., 64:128] = f32r
    # residual — W_r + W_res == W up to ~2^-24.
    wcat = consts.tile([128, NC, 2 * E], F32R, tag="wcat")

    # Issue the W DMA first (ACT HWDGE queue): its 1MB shares the DMA fabric
    # with the x loads below, and W chunk 0 gates the whole wcat setup chain.
    w_sb = consts.tile([64, D], F32, tag="w_sb")
    for q in range(4):
        nc.scalar.dma_start(
            w_sb[:, q * (D // 4):(q + 1) * (D // 4)],
            w_d[:, q * (D // 4):(q + 1) * (D // 4)],
        )

    # Hoist the first two groups' x loads ahead of the W-split setup so the
    # SP HWDGE queue starts streaming x immediately (the setup chain below
    # runs on PE/ACT/DVE and overlaps the transfers).
    x_prefetch = {}

    def issue_x_loads(g, tile0, NT):
        n_splits = 8 if g == 0 else (4 if g == 1 else 2)
        tiles = []
        for j in range(NT):
            xt = sb.tile([128, D], F32, tag="x", bufs=8, name=f"x_{g}_{j}")
            tiles.append(xt)
        dq = D // n_splits
        for q in range(n_splits):
            for j in range(NT):
                r0 = (tile0 + j) * 128
                nc.sync.dma_start(
                    tiles[j][:, q * dq:(q + 1) * dq],
                    x_d[r0:r0 + 128, q * dq:(q + 1) * dq],
                )
        return tiles

    # W-split setup, pipelined per 8-chunk quad so wcat[:, 0:8, :] is ready
    # before the first gating matmul needs it: PE transposes a quad, ACT
    # rounds it into the W_r half of wcat, DVE subtracts (reading the fp32
    # transpose straight from PSUM) into the residual half.
    def setup_w_quad(cq):
        wt_ps = psum.tile([128, 512], F32, tag="xt", bufs=4, name="wt_ps")
        for c8 in range(8):
            c = cq * 8 + c8
            nc.tensor.transpose(
                wt_ps[:, c8 * 64:(c8 + 1) * 64],
                w_sb[:, c * 128:(c + 1) * 128],
                ident64[:],
            )
        qs = slice(cq * 8, (cq + 1) * 8)
        wt_v = wt_ps[:].rearrange("p (c e) -> p c e", e=E)
        nc.scalar.copy(wcat[:, qs, 0:E], wt_v)
        nc.vector.tensor_tensor(
            wcat[:, qs, E:2 * E], wt_v,
            wcat[:, qs, 0:E].bitcast(F32), op=ALU.subtract,
        )

    setup_w_quad(0)
    if SCHED_OVERRIDE is None and n_tokens == 4096:
        x_prefetch[0] = issue_x_loads(0, 0, 4)
        x_prefetch[1] = issue_x_loads(1, 4, 4)
    for cq in range(1, NC // 8):
        setup_w_quad(cq)

    def topk_batch_gen(lp, tile0, G):
        # lp: [128, G, E] logits for token-tiles [tile0, tile0+G).
        # Generator: emission is woven into the next group's chunk loop so
        # the DVE/ACT epilogue ops don't sit ahead of that group's x_res
        # subtracts in the in-order engine queues.
        exp_t = sb.tile([128, G, E], F32, tag="exp" + str(G), bufs=2, name="exp_t")
        den = sb.tile([128, G], F32, tag="den" + str(G), bufs=2, name="den")
        for j in range(G):
            # exp of all 64 logits with fused free-dim sum -> denominator
            nc.scalar.activation(exp_t[:, j, :], lp[:, j, :], EXP,
                                 accum_out=den[:, j:j + 1])
            if j % 4 == 3:
                yield

        # Hardware top-8: Max8 datapath returns the 8 largest values per
        # partition in descending order; max_index recovers their indices
        # (duplicate values get distinct, ascending indices).
        mxs = sb.tile([128, G, K], F32, tag="mxs" + str(G), bufs=2, name="mxs")
        idxi = sb.tile([128, G, K], U32, tag="idxi" + str(G), bufs=2, name="idxi")
        for j in range(G):
            nc.vector.max(mxs[:, j, :], lp[:, j, :])
            nc.vector.max_index(idxi[:, j, :], mxs[:, j, :], lp[:, j, :])
            if j % 3 == 2:
                yield

        rden = sb.tile([128, G], F32, tag="rden" + str(G), bufs=2, name="rden")
        nc.vector.reciprocal(rden[:], den[:])
        exk = sb.tile([128, G, K], F32, tag="exk" + str(G), bufs=2, name="exk")
        nc.scalar.activation(exk[:], mxs[:], EXP)
        vals = sb.tile([128, G, K], F32, tag="vals" + str(G), bufs=2, name="vals")
        rden_b = rden[:].unsqueeze(2).broadcast_to([128, G, K])
        nc.vector.tensor_tensor(vals[:], exk[:], rden_b, op=ALU.mult)
        yield

        # outputs are partition-major [128, T/128, K] in DRAM (contiguous
        # 32B*G runs per partition, descriptor-efficient); host un-permutes.
        nc.scalar.dma_start(vals_d[:, tile0:tile0 + G, :], vals[:])
        nc.scalar.dma_start(idx_d[:, tile0:tile0 + G, :], idxi[:].bitcast(I32))
        yield

    # Group schedule: (n_tiles, flush). First six groups pair into 8-tile
    # topk batches (efficient shape); the tail shrinks to 2/1/1-tile groups
    # so the final topk batch after the last matmul is as small as possible.
    if SCHED_OVERRIDE is not None:
        SCHED = SCHED_OVERRIDE
    elif n_tokens == 4096:
        SCHED = [(4, False), (4, True), (4, False), (4, True), (4, False),
                 (4, True), (4, True), (2, True), (1, True), (1, True)]
    else:
        SCHED = []
        rem = n_tokens // 128
        while rem:
            gsz = min(4, rem)
            SCHED.append((gsz, True))
            rem -= gsz
    assert sum(s for s, _ in SCHED) * 128 == n_tokens

    # Per-group epilogue, emitted as a generator woven into the NEXT group's
    # chunk loop: the fold matmuls then interleave with that group's
    # transposes in the in-order PE queue instead of stalling it at the
    # group boundary on the ACT lgT copy, and the topk/softmax chain drips
    # into the ACT/DVE queues between x_r/x_res ops.
    state = {"lp": None, "lp_pos": 0, "batch_tile0": 0}

    def epilogue_gen(lgT_ps, W_, NT, flush, tile0_g):
        lgT_sb = sb.tile([128, 512], F32, tag="lgTsb", bufs=2, name="lgT_sb")
        nc.scalar.copy(lgT_sb[:, 0:W_], lgT_ps[:, 0:W_])
        yield

        if state["lp"] is None:
            bsz = NT if flush else NT + 4  # batch size: this group (+pair)
            state["lp"] = sb.tile([128, bsz, E], F32, tag=f"lp{bsz}", bufs=2,
                                  name=f"lp{bsz}")
            state["lp_pos"] = 0
            state["batch_tile0"] = tile0_g
        lp = state["lp"]
        for j in range(NT):
            fold_ps = psum.tile([128, E], F32, tag="fold", bufs=2,
                                name="fold_ps")
            nc.tensor.matmul(
                fold_ps[:, :], lgT_sb[:, j * 128:(j + 1) * 128], fm[:],
                start=True, stop=True,
            )
            nc.scalar.copy(lp[:, state["lp_pos"] + j, :], fold_ps[:, :])
            if j % 2 == 1:
                yield
        state["lp_pos"] += NT
        if flush:
            state["lp"] = None
            yield from topk_batch_gen(lp, state["batch_tile0"],
                                      state["lp_pos"])

    pending = None   # in-flight epilogue/topk emission generator (woven)
    _DONE = object()
    tile0 = 0        # global tile index of the current group's first tile
    for g, (NT, flush) in enumerate(SCHED):
        W_ = NT * 128  # active token columns this group
        if X_REUSE and g > 0:
            x_tiles = _x_keep[:NT]
        elif g in x_prefetch:
            x_tiles = x_prefetch.pop(g)
        else:
            # Split loads along d so transposes wait on sub-MB DMA quanta
            # instead of whole 2MB tiles (DMA/PE overlap on HW).
            x_tiles = issue_x_loads(g, tile0, NT)
        if g == 0:
            _x_keep = list(x_tiles)

        # expert-major logitsT accumulator: rows 0:64 = W_r part, 64:128 =
        # W_res part. One bank; bufs=2 decouples consecutive groups.
        lgT_ps = psum.tile([128, 512], F32, tag="lgT", bufs=2, name="lgT_ps")

        def gating(c, bufs_pair):
            xt_r_c, xres_c = bufs_pair
            nc.tensor.matmul(
                lgT_ps[:, 0:W_], wcat[:, c, :], xt_r_c[:, 0:W_],
                start=(c == 0), stop=False, skip_group_check=True,
            )
            nc.tensor.matmul(
                lgT_ps[:, 0:W_], wcat[:, c, :], xres_c[:, 0:W_],
                start=False, stop=(c == NC - 1), skip_group_check=True,
            )

        # software pipeline: gating for chunk c-SKEW is emitted between the
        # transposes of chunk c so the in-order PE stream never waits on the
        # ACT/DVE production of its own chunk's x_r/x_res.
        SKEW = 3
        xt_bufs = {}
        for c in range(NC):
            xt_ps = psum.tile([128, 512], F32, tag="xt", bufs=4, name="xt_ps")
            for j in range(NT):
                nc.tensor.transpose(
                    xt_ps[:, j * 128:(j + 1) * 128],
                    x_tiles[j][:, c * 128:(c + 1) * 128],
                    ident128[:],
                )
            xt_r = sb.tile([128, 512], F32R, tag="xtr", bufs=5, name="xt_r")
            nc.scalar.copy(xt_r[:, 0:W_], xt_ps[:, 0:W_])
            xres = sb.tile([128, 512], F32R, tag="xrs", bufs=5, name="xres")
            nc.vector.tensor_tensor(
                xres[:, 0:W_], xt_ps[:, 0:W_], xt_r[:, 0:W_].bitcast(F32),
                op=ALU.subtract,
            )
            xt_bufs[c] = (xt_r, xres)
            if c >= SKEW:
                gating(c - SKEW, xt_bufs.pop(c - SKEW))
            if pending is not None and c % 3 == 2:
                if next(pending, _DONE) is _DONE:
                    pending = None
        for c in range(NC - SKEW, NC):
            gating(c, xt_bufs.pop(c))

        gen = epilogue_gen(lgT_ps, W_, NT, flush, tile0)
        if pending is None:
            pending = gen
        else:
            # rare: previous epilogue not fully drained — chain them
            prev = pending

            def _chain(a, b):
                yield from a
                yield from b
            pending = _chain(prev, gen)
        if g == len(SCHED) - 1:
            while pending is not None:
                if next(pending, _DONE) is _DONE:
                    pending = None
        tile0 += NT
    while pending is not None:
        if next(pending, _DONE) is _DONE:
            pending = None


_MODEL_CACHE = {}


def build_model(n_tokens=T):
    if n_tokens in _MODEL_CACHE:
        return _MODEL_CACHE[n_tokens]
    nc = bacc.Bacc(
        "TRN2",
        target_bir_lowering=False,
        debug=False,
        enable_asserts=False,
        num_devices=N_CORES,
    )
    x_d = nc.dram_tensor("x", [n_tokens, D], F32, kind="ExternalInput").ap()
    w_d = nc.dram_tensor("w", [E, D], F32, kind="ExternalInput").ap()
    vals_d = nc.dram_tensor(
        "vals", [128, n_tokens // 128, K], F32, kind="ExternalOutput").ap()
    idx_d = nc.dram_tensor(
        "idx", [128, n_tokens // 128, K], I32, kind="ExternalOutput").ap()
    with tile.TileContext(nc) as tc:
        _moe_gate_kernel(tc, [vals_d, idx_d], [x_d, w_d], n_tokens=n_tokens)
    nc.compile()
    _MODEL_CACHE[n_tokens] = nc
    return nc


def run_on_cores(x, W_g, trace=False, trace_kwargs=None):
    """x [32768, 4096] f32, W_g [64, 4096] f32 -> (vals, idx), plus results obj."""
    nc = build_model()
    x = np.ascontiguousarray(np.asarray(x, dtype=np.float32))
    W_g = np.ascontiguousarray(np.asarray(W_g, dtype=np.float32))
    shards = np.split(x, N_CORES, axis=0)
    in_maps = [{"x": shards[i], "w": W_g} for i in range(N_CORES)]
    res = run_bass_kernel_spmd(
        nc, in_maps, core_ids=list(range(N_CORES)), trace=trace,
        **(trace_kwargs or {}),
    )
    # device layout is [128, T/128, K] partition-major; token = tile*128 + p
    vals = np.concatenate(
        [r["vals"].transpose(1, 0, 2).reshape(T, K) for r in res.results], axis=0)
    idx = np.concatenate(
        [r["idx"].transpose(1, 0, 2).reshape(T, K) for r in res.results], axis=0)
    return (vals, idx), res


def kernel(x, W_g):
    (vals, idx), _ = run_on_cores(x, W_g)
    return vals, idx


# revision 13
# speedup vs baseline: 1.4725x; 1.1087x over previous
"""MoE gating (nn_MoEGate) Trainium2 Bass kernel.

kernel(x, W_g) -> (vals, idx), matching the jax reference:
    logits = x @ W_g.T ; scores = softmax(logits) ; vals, idx = top_k(scores, 8)

Strategy: data-parallel over the token dim — 8 NeuronCores each process a
4096-token shard with W_g replicated; no cross-core communication.

Per-core pipeline (T=4096 tokens, D=4096, E=64 experts, K=8):
  - x streamed in natural [token, d] layout (contiguous DMAs, issued ahead
    of the W-split setup so the SP HWDGE queue is never idle).
  - PE transposes 128x128 blocks of x (fp32, exact) into PSUM.
  - Split gating with scale-matched planes, all accumulating into ONE
    expert-major PSUM bank at a 2^18 product scale:
      * x_r = fp16(xT) (ACT copy, 2-byte -> 2x ACT rate); one fp16 matmul
        per d-chunk against stationary [W_h*2^18 || W_l*2^29] (W split into
        fp16 planes, fp16-exact power-of-2 scales).
      * x_res = fp8e4m3(4096*(xT - x_r)) via one DVE tensor_tensor_reduce
        (subtract+scale fused); one fp8 DoubleRow matmul per chunk-PAIR
        (2 chunks of contraction per pass, 0.5 cyc/row) against
        W8 = fp8(W*64), accumulated into the same bank rows 0:64
        ((x_res*2^12)*(W*2^6) = 2^18*x_res*W matches the fp16 planes).
    Logit error vs fp32 ~4.5e-6 rms -> ~6/262144 top-8 index flips.
  - Epilogue per group: ACT copies logitsT to SBUF; one fp32 PE matmul
    per 128-token tile against a fold matrix FM[r,e] = 2^-18*(r==e) +
    2^-29*(r==64+e) that transposes AND descales/folds the planes.
  - Top-8 via the DVE Max8 datapath: one `max` (top-8 values, sorted
    descending) + one `max_index` (their indices) per 128-token tile;
    ties resolve to ascending indices, matching lax.top_k.
  - Softmax: ACT exp with fused free-dim accumulation (accum_out) gives
    the denominator in the same instruction; DVE reciprocal + scale.
  - Per-group epilogue + topk are generators woven into the next group's
    chunk loop so the in-order PE/ACT/DVE queues never stall on them.
  - Group schedule [4x7, 2, 1, 1] tiles shrinks the post-matmul tail.
"""
from contextlib import ExitStack

import numpy as np

import concourse.bacc as bacc
import concourse.mybir as mybir
import concourse.tile as tile
from concourse._compat import with_exitstack
from concourse.bass_utils import run_bass_kernel_spmd

F32 = mybir.dt.float32
F16 = mybir.dt.float16
F8 = mybir.dt.float8e4
I32 = mybir.dt.int32
U32 = mybir.dt.uint32
AX = mybir.AxisListType
ALU = mybir.AluOpType
EXP = mybir.ActivationFunctionType.Exp
COPY = mybir.ActivationFunctionType.Copy
PM = mybir.MatmulPerfMode

N_CORES = 8
N_TOKENS = 32768
D = 4096
E = 64
K = 8
T = N_TOKENS // N_CORES  # 4096 tokens per core
NC = D // 128            # 32 d-chunks
NP = NC // 2             # 16 chunk-pairs (fp8 DoubleRow granularity)
S_XT = 2.0 ** 12         # x_r / x residual scale (fp16 / fp8 planes)
S_WH = 2.0 ** 6          # W_h plane scale (fp16); product scale 2^18
S_WL = 2.0 ** 17         # W_l plane scale (fp16); product scale 2^29
S_W8 = 2.0 ** 6          # W scale for the fp8 plane; product scale 2^18
SCHED_OVERRIDE = None    # debug hook: explicit [(n_tiles, flush), ...]
X_REUSE = False          # debug hook: skip x DMAs after group 0 (timing)


@with_exitstack
def _moe_gate_kernel(ctx: ExitStack, tc: tile.TileContext, outs, ins, n_tokens=T):
    nc = tc.nc
    x_d, w_d = ins
    vals_d, idx_d = outs

    consts = ctx.enter_context(tc.tile_pool(name="consts", bufs=1))
    psum = ctx.enter_context(tc.tile_pool(name="psum", bufs=1, space="PSUM"))

    ident128 = consts.tile([128, 128], F32, tag="ident128")
    nc.vector.memset(ident128[:], 1.0)
    nc.gpsimd.affine_select(
        ident128[:], ident128[:], pattern=[[-1, 128]], compare_op=ALU.is_equal,
        fill=0.0, base=0, channel_multiplier=1,
    )
    ident64 = consts.tile([64, 64], F32, tag="ident64")
    nc.vector.memset(ident64[:], 1.0)
    nc.gpsimd.affine_select(
        ident64[:], ident64[:], pattern=[[-1, 64]], compare_op=ALU.is_equal,
        fill=0.0, base=0, channel_multiplier=1,
    )
    # Fold matrix FM [128, 64]: FM[r, e] = 2^-18*(r == e) + 2^-29*(r == 64+e);
    # one fp32 matmul lgT_block^T @ FM transposes a 128-token block of
    # expert-major logitsT and folds/descales the W_h / W_l plane halves
    # (the fp8 residual plane accumulates into rows 0:64 at the same 2^18
    # product scale as the W_h half).
    fm = consts.tile([128, E], F32, tag="fm")
    fm_b = consts.tile([128, E], F32, tag="fm_b")
    nc.vector.memset(fm[:], 1.0 / (S_XT * S_WH))
    nc.gpsimd.affine_select(
        fm[:], fm[:], pattern=[[-1, E]], compare_op=ALU.is_equal,
        fill=0.0, base=0, channel_multiplier=1,
    )
    nc.vector.memset(fm_b[:], 1.0 / (S_XT * S_WL))
    nc.gpsimd.affine_select(
        fm_b[:], fm_b[:], pattern=[[-1, E]], compare_op=ALU.is_equal,
        fill=0.0, base=-64, channel_multiplier=1,
    )
    nc.vector.tensor_tensor(fm[:], fm[:], fm_b[:], op=ALU.add)

    sb = ctx.enter_context(tc.tile_pool(name="main", bufs=1))

    # W planes: w16cat[128 d, c, 0:64] = fp16(W^T)*2^18, [.., 64:128] =
    # fp16((W^T - W_h)*2^29); w8[128 d, pair, z, e] = fp8(W^T * 64).
    w16cat = consts.tile([128, NC, 2 * E], F16, tag="w16cat")
    w8 = consts.tile([128, NP, 2, E], F8, tag="w8")

    # Issue the W DMA first (ACT HWDGE queue): its 1MB shares the DMA fabric
    # with the x loads below, and W chunk 0 gates the whole setup chain.
    w_sb = consts.tile([64, D], F32, tag="w_sb")
    for q in range(4):
        nc.scalar.dma_start(
            w_sb[:, q * (D // 4):(q + 1) * (D // 4)],
            w_d[:, q * (D // 4):(q + 1) * (D // 4)],
        )

    # Hoist the first two groups' x loads ahead of the W-split setup so the
    # SP HWDGE queue starts streaming x immediately (the setup chain below
    # runs on PE/ACT/DVE and overlaps the transfers).
    x_prefetch = {}

    def issue_x_loads(g, tile0, NT):
        n_splits = 8 if g == 0 else (4 if g == 1 else 2)
        tiles = []
        for j in range(NT):
            xt = sb.tile([128, D], F32, tag="x", bufs=8, name=f"x_{g}_{j}")
            tiles.append(xt)
        dq = D // n_splits
        for q in range(n_splits):
            for j in range(NT):
                r0 = (tile0 + j) * 128
                nc.sync.dma_start(
                    tiles[j][:, q * dq:(q + 1) * dq],
                    x_d[r0:r0 + 128, q * dq:(q + 1) * dq],
                )
        return tiles

    # W-plane setup, pipelined per 8-chunk quad so the first chunks' planes
    # are ready before the first gating matmuls need them: PE transposes a
    # quad (fp32, exact), ACT casts fp16/fp8 planes, DVE fuses the residual
    # subtract+scale.
    def setup_w_quad(cq):
        wt_ps = psum.tile([128, 512], F32, tag="xt", bufs=4, name="wt_ps")
        for c8 in range(8):
            c = cq * 8 + c8
            nc.tensor.transpose(
                wt_ps[:, c8 * 64:(c8 + 1) * 64],
                w_sb[:, c * 128:(c + 1) * 128],
                ident64[:],
            )
        qs = slice(cq * 8, (cq + 1) * 8)
        wt_v = wt_ps[:].rearrange("p (c e) -> p c e", e=E)
        # wh16 = fp16(W^T * 2^17); power-of-2 rescales of it are exact.
        wh16 = sb.tile([128, 8, E], F16, tag="wh16", bufs=2, name="wh16")
        nc.scalar.activation(wh16[:], wt_v, COPY, scale=S_WL)
        nc.scalar.activation(w16cat[:, qs, 0:E], wh16[:], COPY,
                             scale=S_WH / S_WL)
        # W_l*2^17 = W*2^17 - W_h*2^17, fused scale via scalar_tensor_tensor
        nc.vector.scalar_tensor_tensor(
            w16cat[:, qs, E:2 * E], wt_v, S_WL, wh16[:],
            op0=ALU.mult, op1=ALU.subtract,
        )
        nc.scalar.activation(
            w8[:, 4 * cq:4 * (cq + 1), :, :].rearrange("p q z e -> p (q z) e"),
            wh16[:], COPY, scale=S_W8 / S_WL,
        )

    setup_w_quad(0)
    if SCHED_OVERRIDE is None and n_tokens == 4096:
        x_prefetch[0] = issue_x_loads(0, 0, 4)
        x_prefetch[1] = issue_x_loads(1, 4, 4)
    for cq in range(1, NC // 8):
        setup_w_quad(cq)

    def topk_batch_gen(lp, tile0, G):
        # lp: [128, G, E] logits for token-tiles [tile0, tile0+G).
        # Generator: emission is woven into the next group's chunk loop so
        # the DVE/ACT epilogue ops don't sit ahead of that group's x_res
        # ops in the in-order engine queues.
        exp_t = sb.tile([128, G, E], F32, tag="exp" + str(G), bufs=2, name="exp_t")
        den = sb.tile([128, G], F32, tag="den" + str(G), bufs=2, name="den")
        for j in range(G):
            # exp of all 64 logits with fused free-dim sum -> denominator
            nc.scalar.activation(exp_t[:, j, :], lp[:, j, :], EXP,
                                 accum_out=den[:, j:j + 1])
            if j % 4 == 3:
                yield

        # Hardware top-8: Max8 datapath returns the 8 largest values per
        # partition in descending order; max_index recovers their indices
        # (duplicate values get distinct, ascending indices).
        mxs = sb.tile([128, G, K], F32, tag="mxs" + str(G), bufs=2, name="mxs")
        idxi = sb.tile([128, G, K], U32, tag="idxi" + str(G), bufs=2, name="idxi")
        for j in range(G):
            nc.vector.max(mxs[:, j, :], lp[:, j, :])
            nc.vector.max_index(idxi[:, j, :], mxs[:, j, :], lp[:, j, :])
            if j % 3 == 2:
                yield

        rden = sb.tile([128, G], F32, tag="rden" + str(G), bufs=2, name="rden")
        nc.vector.reciprocal(rden[:], den[:])
        exk = sb.tile([128, G, K], F32, tag="exk" + str(G), bufs=2, name="exk")
        nc.scalar.activation(exk[:], mxs[:], EXP)
        vals = sb.tile([128, G, K], F32, tag="vals" + str(G), bufs=2, name="vals")
        rden_b = rden[:].unsqueeze(2).broadcast_to([128, G, K])
        nc.vector.tensor_tensor(vals[:], exk[:], rden_b, op=ALU.mult)
        yield

        # outputs are partition-major [128, T/128, K] in DRAM (contiguous
        # 32B*G runs per partition, descriptor-efficient); host un-permutes.
        nc.scalar.dma_start(vals_d[:, tile0:tile0 + G, :], vals[:])
        nc.scalar.dma_start(idx_d[:, tile0:tile0 + G, :], idxi[:].bitcast(I32))
        yield

    # Group schedule: (n_tiles, flush). First six groups pair into 8-tile
    # topk batches (efficient shape); the tail shrinks to 2/1/1-tile groups
    # so the final topk batch after the last matmul is as small as possible.
    if SCHED_OVERRIDE is not None:
        SCHED = SCHED_OVERRIDE
    elif n_tokens == 4096:
        SCHED = [(4, False), (4, True), (4, False), (4, True), (4, False),
                 (4, True), (4, True), (2, True), (1, True), (1, True)]
    else:
        SCHED = []
        rem = n_tokens // 128
        while rem:
            gsz = min(4, rem)
            SCHED.append((gsz, True))
            rem -= gsz
    assert sum(s for s, _ in SCHED) * 128 == n_tokens

    # Per-group epilogue, emitted as a generator woven into the NEXT group's
    # chunk loop: the fold matmuls then interleave with that group's
    # transposes in the in-order PE queue instead of stalling it at the
    # group boundary on the ACT lgT copy, and the topk/softmax chain drips
    # into the ACT/DVE queues between x_r/x_res ops.
    state = {"lp": None, "lp_pos": 0, "batch_tile0": 0}

    def epilogue_gen(lgT_ps, W_, NT, flush, tile0_g):
        lgT_sb = sb.tile([128, 512], F32, tag="lgTsb", bufs=2, name="lgT_sb")
        nc.scalar.copy(lgT_sb[:, 0:W_], lgT_ps[:, 0:W_])
        yield

        if state["lp"] is None:
            bsz = NT if flush else NT + 4  # batch size: this group (+pair)
            state["lp"] = sb.tile([128, bsz, E], F32, tag=f"lp{bsz}", bufs=2,
                                  name=f"lp{bsz}")
            state["lp_pos"] = 0
            state["batch_tile0"] = tile0_g
        lp = state["lp"]
        for j in range(NT):
            fold_ps = psum.tile([128, E], F32, tag="fold", bufs=2,
                                name="fold_ps")
            nc.tensor.matmul(
                fold_ps[:, :], lgT_sb[:, j * 128:(j + 1) * 128], fm[:],
                start=True, stop=True,
            )
            nc.scalar.copy(lp[:, state["lp_pos"] + j, :], fold_ps[:, :])
            if j % 2 == 1:
                yield
        state["lp_pos"] += NT
        if flush:
            state["lp"] = None
            yield from topk_batch_gen(lp, state["batch_tile0"],
                                      state["lp_pos"])

    pending = None   # in-flight epilogue/topk emission generator (woven)
    _DONE = object()
    tile0 = 0        # global tile index of the current group's first tile
    for g, (NT, flush) in enumerate(SCHED):
        W_ = NT * 128  # active token columns this group
        if X_REUSE and g > 0:
            x_tiles = _x_keep[:NT]
        elif g in x_prefetch:
            x_tiles = x_prefetch.pop(g)
        else:
            # Split loads along d so transposes wait on sub-MB DMA quanta
            # instead of whole 2MB tiles (DMA/PE overlap on HW).
            x_tiles = issue_x_loads(g, tile0, NT)
        if g == 0:
            _x_keep = list(x_tiles)

        # expert-major logitsT accumulator at 2^18 product scale: rows
        # 0:64 = W_h plane + fp8 residual plane, 64:128 = W_l plane.
        # One bank; bufs=2 decouples consecutive groups.
        lgT_ps = psum.tile([128, 512], F32, tag="lgT", bufs=2, name="lgT_ps")

        def gating(c, xt_r_c, pair_tile):
            nc.tensor.matmul(
                lgT_ps[:, 0:W_], w16cat[:, c, :], xt_r_c[:, 0:W_],
                start=(c == 0), stop=False, skip_group_check=True,
            )
            if c % 2 == 1:
                q = c // 2
                nc.tensor.matmul(
                    lgT_ps[0:64, 0:W_], w8[:, q, :, :],
                    pair_tile[:, :, 0:W_],
                    start=False, stop=(q == NP - 1),
                    perf_mode=PM.DoubleRow, skip_group_check=True,
                )

        # software pipeline: gating for chunk c-SKEW is emitted between the
        # transposes of chunk c so the in-order PE stream never waits on the
        # ACT/DVE production of its own chunk's x_r/x_res.
        SKEW = 3
        xt_bufs = {}
        pair8 = None
        for c in range(NC):
            xt_ps = psum.tile([128, 512], F32, tag="xt", bufs=4, name="xt_ps")
            for j in range(NT):
                nc.tensor.transpose(
                    xt_ps[:, j * 128:(j + 1) * 128],
                    x_tiles[j][:, c * 128:(c + 1) * 128],
                    ident128[:],
                )
            # x_r = fp16(xT * 2^12) — the scale rides the ACT copy for free
            xt_r = sb.tile([128, 512], F16, tag="xtr", bufs=5, name="xt_r")
            nc.scalar.activation(xt_r[:, 0:W_], xt_ps[:, 0:W_], COPY,
                                 scale=S_XT)
            if c % 2 == 0:
                pair8 = sb.tile([128, 2, 512], F8, tag="xr8", bufs=3,
                                name="xres8")
            # fused residual: x_res8 = fp8e4m3(xT*2^12 - x_r*2^12)
            nc.vector.scalar_tensor_tensor(
                pair8[:, c % 2, 0:W_], xt_ps[:, 0:W_], S_XT, xt_r[:, 0:W_],
                op0=ALU.mult, op1=ALU.subtract,
            )
            xt_bufs[c] = (xt_r, pair8)
            if c >= SKEW:
                gating(c - SKEW, *xt_bufs.pop(c - SKEW))
            if pending is not None and c % 3 == 2:
                if next(pending, _DONE) is _DONE:
                    pending = None
        for c in range(NC - SKEW, NC):
            gating(c, *xt_bufs.pop(c))

        gen = epilogue_gen(lgT_ps, W_, NT, flush, tile0)
        if pending is None:
            pending = gen
        else:
            # rare: previous epilogue not fully drained — chain them
            prev = pending

            def _chain(a, b):
                yield from a
                yield from b
            pending = _chain(prev, gen)
        if g == len(SCHED) - 1:
            while pending is not None:
                if next(pending, _DONE) is _DONE:
                    pending = None
        tile0 += NT
    while pending is not None:
        if next(pending, _DONE) is _DONE:
            pending = None


_MODEL_CACHE = {}


def build_model(n_tokens=T):
    if n_tokens in _MODEL_CACHE:
        return _MODEL_CACHE[n_tokens]
    nc = bacc.Bacc(
        "TRN2",
        target_bir_lowering=False,
        debug=False,
        enable_asserts=False,
        num_devices=N_CORES,
    )
    x_d = nc.dram_tensor("x", [n_tokens, D], F32, kind="ExternalInput").ap()
    w_d = nc.dram_tensor("w", [E, D], F32, kind="ExternalInput").ap()
    vals_d = nc.dram_tensor(
        "vals", [128, n_tokens // 128, K], F32, kind="ExternalOutput").ap()
    idx_d = nc.dram_tensor(
        "idx", [128, n_tokens // 128, K], I32, kind="ExternalOutput").ap()
    with tile.TileContext(nc) as tc:
        _moe_gate_kernel(tc, [vals_d, idx_d], [x_d, w_d], n_tokens=n_tokens)
    nc.compile()
    _MODEL_CACHE[n_tokens] = nc
    return nc


def run_on_cores(x, W_g, trace=False, trace_kwargs=None):
    """x [32768, 4096] f32, W_g [64, 4096] f32 -> (vals, idx), plus results obj."""
    nc = build_model()
    x = np.ascontiguousarray(np.asarray(x, dtype=np.float32))
    W_g = np.ascontiguousarray(np.asarray(W_g, dtype=np.float32))
    shards = np.split(x, N_CORES, axis=0)
    in_maps = [{"x": shards[i], "w": W_g} for i in range(N_CORES)]
    res = run_bass_kernel_spmd(
        nc, in_maps, core_ids=list(range(N_CORES)), trace=trace,
        **(trace_kwargs or {}),
    )
    # device layout is [128, T/128, K] partition-major; token = tile*128 + p
    vals = np.concatenate(
        [r["vals"].transpose(1, 0, 2).reshape(T, K) for r in res.results], axis=0)
    idx = np.concatenate(
        [r["idx"].transpose(1, 0, 2).reshape(T, K) for r in res.results], axis=0)
    return (vals, idx), res


def kernel(x, W_g):
    (vals, idx), _ = run_on_cores(x, W_g)
    return vals, idx
